# revision 1
# baseline (speedup 1.0000x reference)
"""Fused attention block (qkv proj + pooled attention + 16-head masked
attention + out proj) for TRN2, batch-parallel across 8 NeuronCores.

Layouts per core (batch element b):
  xT      [D=1024, n=1024]   x transposed (host-side), feature-major
  qT,kT   [f, i]  feature-major  (8 tiles each [128, 1024])
  v'      [j, 65*16]  position-major, per-head 64 cols + a ones column
  P^T     [j, i]  unnormalized exp(attention) transposed
  aoT     [f, i]  un/normalized head outputs, feature-major
Outputs: out [n, D] natural, attn_ [n, n] natural.

Masked softmax without max-subtraction (args are small, exp safe):
  key mask  -> additive -80 bias on exp (per j-tile partition bias)
  query mask-> rows fixed at the out-projection via a rank-2 correction
               matmul adding  ones*b_out + (1-qkeep)*ybar  after zeroing
               masked columns through the normalization factor.
"""
import os
import sys

sys.path.insert(0, "/opt/trn_rl_repo")

import numpy as np

import concourse.bass as bass
import concourse.mybir as mybir
import concourse.tile as tile
from concourse import bacc, bass_utils

F32 = mybir.dt.float32
F32R = mybir.dt.float32r
EXP = mybir.ActivationFunctionType.Exp

B = 8
N = 1024          # sequence (after CLS pad)
D = 1024          # model dim
H = 16
DH = 64
NT = N // 128     # 8 row tiles
SCALE_H = DH ** -0.5     # 1/8
SCALE_P = D ** -0.5      # 1/32
NEG = -80.0

_CACHED = {}


def build_nc():
    nc = bacc.Bacc("TRN2", target_bir_lowering=False, debug=False, num_devices=8)
    xT = nc.dram_tensor("xT", [D, N], F32R, kind="ExternalInput").ap()
    wqkp = nc.dram_tensor("wqkp", [D, 2 * D], F32R, kind="ExternalInput").ap()
    wv_in = nc.dram_tensor("wv_in", [D, D], F32R, kind="ExternalInput").ap()
    wout = nc.dram_tensor("wout", [D, D], F32R, kind="ExternalInput").ap()
    bout = nc.dram_tensor("bout", [1, D], F32R, kind="ExternalInput").ap()
    cmaskT = nc.dram_tensor("cmaskT", [128, NT], F32, kind="ExternalInput").ap()
    qkeep16 = nc.dram_tensor("qkeep16", [H, N], F32, kind="ExternalInput").ap()
    fixl_in = nc.dram_tensor("fixl_in", [2, N], F32R, kind="ExternalInput").ap()
    sel_in = nc.dram_tensor("sel_in", [H, D], F32R, kind="ExternalInput").ap()
    vones_in = nc.dram_tensor("vones_in", [128, H], F32R, kind="ExternalInput").ap()
    out_d = nc.dram_tensor("out", [N, D], F32, kind="ExternalOutput").ap()
    attn_d = nc.dram_tensor("attn", [N, N], F32, kind="ExternalOutput").ap()

    with tile.TileContext(nc, trace_sim=bool(os.environ.get('ATTN_TRACE_SIM'))) as tc:
        with (
            tc.tile_pool(name="big8", bufs=8) as big8,      # xT then aoT (slot reuse)
            tc.tile_pool(name="qk", bufs=16) as qkp,
            tc.tile_pool(name="vp", bufs=8) as vpp,
            tc.tile_pool(name="pt", bufs=3) as ptp,
            tc.tile_pool(name="wq", bufs=3) as wqp,
            tc.tile_pool(name="wv", bufs=3) as wvp,
            tc.tile_pool(name="wo", bufs=2) as wop,
            tc.tile_pool(name="outt", bufs=2) as outp,
            tc.tile_pool(name="one", bufs=1) as onep,
            tc.tile_pool(name="small", bufs=4) as smallp,
            tc.tile_pool(name="mm", bufs=2, space="PSUM") as mmp,
            tc.tile_pool(name="av", bufs=2, space="PSUM") as avp,
        ):
            # ---- constants ----
            cmask_t = onep.tile([128, NT], F32, name="cmask_t", tag="cmask_t")
            nc.gpsimd.dma_start(out=cmask_t, in_=cmaskT)
            qkeep_t = onep.tile([H, N], F32, name="qkeep_t", tag="qkeep_t")
            nc.gpsimd.dma_start(out=qkeep_t, in_=qkeep16)
            fixl1 = onep.tile([1, N], F32R, name="fixl1", tag="fixl1")
            nc.gpsimd.dma_start(out=fixl1, in_=fixl_in[0:1, :])
            fixl2 = onep.tile([1, N], F32R, name="fixl2", tag="fixl2")
            nc.gpsimd.dma_start(out=fixl2, in_=fixl_in[1:2, :])
            sel = onep.tile([H, D], F32R, name="sel", tag="sel")
            nc.gpsimd.dma_start(out=sel, in_=sel_in)
            fixr1 = onep.tile([1, N], F32R, name="fixr1", tag="fixr1")
            nc.gpsimd.dma_start(out=fixr1, in_=bout)
            fixr2 = onep.tile([1, N], F32R, name="fixr2", tag="fixr2")
            onescol = onep.tile([128, 1], F32, name="onescol", tag="onescol")
            nc.vector.memset(onescol, 1.0 / N)
            vbarT = onep.tile([128, NT], F32R, name="vbarT", tag="vbarT")
            srecraw = onep.tile([H, N], F32, name="srecraw", tag="srecraw")
            srec = onep.tile([H, N], F32, name="srec", tag="srec")
            sq = onep.tile([H, N], F32R, name="sq", tag="sq")

            # ---- load xT ----
            xts = []
            for t in range(NT):
                xt = big8.tile([128, N], F32R, tag="big", name=f"xt{t}")
                nc.sync.dma_start(out=xt, in_=xT[t * 128:(t + 1) * 128, :])
                xts.append(xt)

            # ---- V position-major (v' with per-head ones column) ----
            v_tiles = []
            for t in range(NT):
                vt = vpp.tile([128, 65 * H], F32R, tag="v", name=f"v{t}")
                v3 = vt.rearrange("p (h d) -> p h d", d=65)
                nc.gpsimd.dma_start(out=v3[:, :, 64:65], in_=vones_in[:, :, None])
                v_tiles.append(vt)
            for pass_ in range(2):
                pss = []
                for q4 in range(4):
                    pool = mmp if q4 < 2 else avp
                    ps = pool.tile([128, N], F32, tag="mm" if q4 < 2 else "av",
                                   name=f"ps_v{pass_}{q4}")
                    pss.append(ps)
                for kt in range(NT):
                    for c in range(2):
                        w = wvp.tile([128, 512], F32R, tag="wv", name="w_v")
                        nc.sync.dma_start(
                            out=w,
                            in_=wv_in[kt * 128:(kt + 1) * 128,
                                      c * 512:(c + 1) * 512],
                        )
                        for q4 in range(4):
                            it = pass_ * 4 + q4
                            nc.tensor.matmul(
                                pss[q4][:, c * 512:(c + 1) * 512],
                                xts[kt][:, it * 128:(it + 1) * 128],
                                w[:],
                                start=(kt == 0),
                                stop=(kt == NT - 1),
                            )
                for q4 in range(4):
                    it = pass_ * 4 + q4
                    v3 = v_tiles[it].rearrange("p (h d) -> p h d", d=65)
                    for c in range(2):
                        src = pss[q4][:, c * 512:(c + 1) * 512].rearrange(
                            "p (h d) -> p h d", d=64)
                        nc.vector.tensor_copy(v3[:, c * 8:(c + 1) * 8, 0:64], src)

            # ---- QKV: qT,kT feature-major (head-pair emission order) ----
            qk_tiles = [None] * (2 * NT)
            for fpair in range(NT):
                pair_ps = []
                for half in (0, 1):
                    ft = fpair + half * NT
                    pool = mmp if half == 0 else avp
                    ps = pool.tile([128, N], F32, tag="mm" if half == 0 else "av",
                                   name=f"ps_qk{ft}")
                    pair_ps.append(ps)
                for kt in range(NT):
                    w = wqp.tile([128, 256], F32R, tag="wq", name="w_qk")
                    nc.sync.dma_start(
                        out=w,
                        in_=wqkp[kt * 128:(kt + 1) * 128,
                                 fpair * 256:(fpair + 1) * 256])
                    for half in (0, 1):
                        for c in range(2):
                            nc.tensor.matmul(
                                pair_ps[half][:, c * 512:(c + 1) * 512],
                                w[:, half * 128:(half + 1) * 128],
                                xts[kt][:, c * 512:(c + 1) * 512],
                                start=(kt == 0),
                                stop=(kt == NT - 1),
                            )
                for half in (0, 1):
                    ft = fpair + half * NT
                    qk = qkp.tile([128, N], F32R, tag="qk", name=f"qk{ft}")
                    nc.vector.tensor_copy(qk, pair_ps[half])
                    qk_tiles[ft] = qk

                # vbar f-tile for this pair (fills DMA gaps; needs only v')
                t = fpair
                psv = mmp.tile([128, N], F32, tag="mm", name=f"ps_vb{t}")
                for jt in range(NT):
                    for u in range(2):
                        h = 2 * t + u
                        nc.tensor.matmul(
                            psv[u * 64:(u + 1) * 64, 0:1],
                            v_tiles[jt][:, h * 65:h * 65 + 64].bitcast(F32),
                            onescol[:],
                            start=(jt == 0), stop=(jt == NT - 1),
                        )
                nc.vector.tensor_copy(vbarT[:, t:t + 1], psv[:, 0:1])

            # ---- heads (paired for row-tiling concurrency) + pooled ----
            ao_tiles = []
            for ft in range(NT):
                ao = big8.tile([128, N], F32R, tag="big", name=f"ao{ft}")
                ao_tiles.append(ao)

            for hp in range(NT):  # head pair index = f-tile of qk
                avs = []
                for u in range(2):
                    a = avp.tile([128, N], F32, tag="av", name=f"av{hp}{u}")
                    avs.append(a)
                for jt in range(NT):
                    dps = []
                    for u in range(2):
                        h = 2 * hp + u
                        off = u * 64
                        dp = mmp.tile([128, N], F32, tag="mm", name=f"dp{h}{jt}")
                        kt_ap = qk_tiles[NT + hp][off:off + 64,
                                                  jt * 128:(jt + 1) * 128]
                        for c in range(2):
                            qt_ap = qk_tiles[hp][off:off + 64,
                                                 c * 512:(c + 1) * 512]
                            nc.tensor.matmul(
                                dp[:, c * 512:(c + 1) * 512], kt_ap, qt_ap,
                                start=True, stop=True,
                            )
                        dps.append(dp)
                    for u in range(2):
                        h = 2 * hp + u
                        pt = ptp.tile([128, N], F32R, tag="pt", name=f"pt{h}{jt}")
                        nc.scalar.activation(
                            pt, dps[u], EXP,
                            bias=cmask_t[:, jt:jt + 1], scale=SCALE_H,
                        )
                        vh = v_tiles[jt][:, h * 65:(h + 1) * 65]
                        for c in range(2):
                            nc.tensor.matmul(
                                avs[u][0:65, c * 512:(c + 1) * 512],
                                vh, pt[:, c * 512:(c + 1) * 512],
                                start=(jt == 0), stop=(jt == NT - 1),
                            )
                for u in range(2):
                    h = 2 * hp + u
                    nc.vector.tensor_copy(
                        ao_tiles[hp][u * 64:(u + 1) * 64, :], avs[u][0:64, :])
                    stg = smallp.tile([1, N], F32, tag="stg", bufs=1, name=f"stg{h}")
                    nc.vector.tensor_copy(stg, avs[u][64:65, :])
                    nc.gpsimd.dma_start(out=srecraw[h:h + 1, :], in_=stg)

                # pooled attention tile hp (interleaved for ACT/PE overlap)
                it = hp
                ps = mmp.tile([128, N], F32, tag="mm", name=f"ps_pool{it}")
                for ft in range(NT):
                    lhs = qk_tiles[ft][:, it * 128:(it + 1) * 128]
                    for c in range(2):
                        nc.tensor.matmul(
                            ps[:, c * 512:(c + 1) * 512],
                            lhs, qk_tiles[NT + ft][:, c * 512:(c + 1) * 512],
                            start=(ft == 0), stop=(ft == NT - 1),
                        )
                pe = ptp.tile([128, N], F32, tag="pt", name=f"pe{it}")
                sums = smallp.tile([128, 1], F32, tag="sm", name=f"sums{it}")
                nc.scalar.activation(pe, ps, EXP, scale=SCALE_P, accum_out=sums)
                rec = smallp.tile([128, 1], F32, tag="rc", name=f"rec{it}")
                nc.vector.reciprocal(rec, sums)
                ot = outp.tile([128, N], F32, tag="ot", name=f"ot_a{it}")
                nc.vector.tensor_scalar_mul(ot, pe, rec)
                nc.gpsimd.dma_start(out=attn_d[it * 128:(it + 1) * 128, :], in_=ot)

            # ---- normalize head outputs (zero masked-query columns) ----
            nc.vector.reciprocal(srec[:], srecraw[:])
            nc.vector.tensor_mul(sq[:], srec[:], qkeep_t[:])
            for ft in range(NT):
                bp = mmp.tile([128, N], F32, tag="mm", name=f"bp{ft}")
                for c in range(2):
                    nc.tensor.matmul(
                        bp[:, c * 512:(c + 1) * 512],
                        sel[:, ft * 128:(ft + 1) * 128],
                        sq[:, c * 512:(c + 1) * 512],
                        start=True, stop=True,
                    )
                nc.vector.tensor_mul(ao_tiles[ft], ao_tiles[ft], bp)

            # ---- ybar = vbar @ wout (row for masked queries) ----
            ybps = mmp.tile([128, N], F32, tag="mm", name="ybps")
            for ft in range(NT):
                w = wop.tile([128, N], F32R, tag="wo", name="w_y")
                nc.gpsimd.dma_start(out=w, in_=wout[ft * 128:(ft + 1) * 128, :])
                for c in range(2):
                    nc.tensor.matmul(
                        ybps[0:1, c * 512:(c + 1) * 512],
                        vbarT[:, ft:ft + 1],
                        w[:, c * 512:(c + 1) * 512],
                        start=(ft == 0), stop=(ft == NT - 1),
                    )
            nc.vector.tensor_copy(fixr2, ybps[0:1, :])

            # ---- out projection (+rank-2 fix) ----
            for pass_ in range(2):
                pss = []
                for q4 in range(4):
                    pool = mmp if q4 < 2 else avp
                    ps = pool.tile([128, N], F32, tag="mm" if q4 < 2 else "av",
                                   name=f"ps_o{pass_}{q4}")
                    pss.append(ps)
                for ft in range(NT):
                    w = wop.tile([128, N], F32R, tag="wo", name="w_o")
                    nc.gpsimd.dma_start(
                        out=w, in_=wout[ft * 128:(ft + 1) * 128, :])
                    for c in range(2):
                        for q4 in range(4):
                            it = pass_ * 4 + q4
                            nc.tensor.matmul(
                                pss[q4][:, c * 512:(c + 1) * 512],
                                ao_tiles[ft][:, it * 128:(it + 1) * 128],
                                w[:, c * 512:(c + 1) * 512],
                                start=(ft == 0), stop=False,
                            )
                for q4 in range(4):
                    it = pass_ * 4 + q4
                    for c in range(2):
                        nc.tensor.matmul(
                            pss[q4][:, c * 512:(c + 1) * 512],
                            fixl1[:, it * 128:(it + 1) * 128],
                            fixr1[:, c * 512:(c + 1) * 512],
                            start=False, stop=False,
                        )
                        nc.tensor.matmul(
                            pss[q4][:, c * 512:(c + 1) * 512],
                            fixl2[:, it * 128:(it + 1) * 128],
                            fixr2[:, c * 512:(c + 1) * 512],
                            start=False, stop=True,
                        )
                    ot = outp.tile([128, N], F32, tag="ot", name=f"ot_o{it}")
                    nc.vector.tensor_copy(ot, pss[q4])
                    nc.sync.dma_start(
                        out=out_d[it * 128:(it + 1) * 128, :], in_=ot)

    nc.compile()
    return nc


def _host_prep(x, mask, w_qkv, w_out, b_out):
    in_maps = []
    sel = np.zeros((H, D), dtype=np.float32)
    sel[np.arange(D) // DH, np.arange(D)] = 1.0
    wq3 = w_qkv.reshape(D, 3, NT, 128)
    wqkp = np.ascontiguousarray(
        np.stack([wq3[:, 0], wq3[:, 1]], axis=2).reshape(D, 2 * D))
    wv = np.ascontiguousarray(w_qkv[:, 2 * D:])
    for b in range(B):
        m = np.concatenate([[True], mask[b]]).astype(np.float32)  # [N]
        cm = np.where(m > 0, 0.0, NEG).astype(np.float32)
        in_maps.append({
            "xT": np.ascontiguousarray(x[b].T),
            "wqkp": wqkp,
            "wv_in": wv,
            "wout": w_out,
            "bout": b_out.reshape(1, D),
            "cmaskT": np.ascontiguousarray(cm.reshape(NT, 128).T),
            "qkeep16": np.tile(m, (H, 1)),
            "fixl_in": np.stack([np.ones(N, np.float32), 1.0 - m]),
            "sel_in": sel,
            "vones_in": np.ones((128, H), np.float32),
        })
    return in_maps


def kernel(x, mask, w_qkv, w_out, b_out, **run_kw):
    if "nc" not in _CACHED:
        _CACHED["nc"] = build_nc()
    nc = _CACHED["nc"]
    in_maps = _host_prep(
        np.asarray(x, np.float32), np.asarray(mask),
        np.asarray(w_qkv, np.float32), np.asarray(w_out, np.float32),
        np.asarray(b_out, np.float32))
    try:
        res = bass_utils.run_bass_kernel_spmd(
            nc, in_maps, core_ids=list(range(B)), **run_kw)
    except Exception:
        # transient NRT device wedge: retry once
        res = bass_utils.run_bass_kernel_spmd(
            nc, in_maps, core_ids=list(range(B)), **run_kw)
    out = np.stack([res.results[b]["out"] for b in range(B)])
    attn_ = np.stack([res.results[b]["attn"] for b in range(B)])
    _CACHED["last_results"] = res
    return out, attn_



# revision 16
# speedup vs baseline: 1.3006x; 1.3006x over previous
"""Fused attention block (qkv proj + pooled attention + 16-head masked
attention + out proj) for TRN2, batch-parallel across 8 NeuronCores.

Key optimizations vs the straightforward version:
  - Key/value compaction: masked keys (~50%) are dropped on the host by
    gathering kept columns of x^T into xcT [D, 640] (padded with CLS,
    pad slots get a -80 exp bias). V-proj, head dots, exp, and attn@V
    all shrink by ~3/8. The pooled attention is unmasked and keeps the
    full K.
  - attn@V is "flipped": stationary = exp(dots) [jc, i-block], moving =
    per-head V plus a ones column [jc, 65] in bf16 (bf16 keeps 1
    cycle/row at a 65-wide moving operand). Output lands i-major with
    the softmax denominator in the 65th column -> per-partition
    normalization on DVE, then a PE transpose to feature-major aoT.
  - Masked queries are zeroed via a per-partition qkeep multiply and
    fixed at the out-projection by a single K=2 rank-2 correction
    matmul (ones x bout + (1-m)/N x ybar), ybar computed on the host.
  - Out projection is split over d-halves so each half's PSUM fits,
    with one bf16 wout stream.
  - Software pipelining: QK-proj of pair F+1 is emitted before
    dots/exp/attn@V of pair F so the ACT-engine exp stretch overlaps
    the next projection's matmuls.
  - DMAs spread across SP/Activation/Pool queues.
"""
import os
import sys
from contextlib import ExitStack

sys.path.insert(0, "/opt/trn_rl_repo")

import numpy as np

import concourse.bass as bass
import concourse.mybir as mybir
import concourse.tile as tile
from concourse import bacc, bass_utils

F32 = mybir.dt.float32
F32R = mybir.dt.float32r
BF16 = mybir.dt.bfloat16
EXP = mybir.ActivationFunctionType.Exp

B = 8
N = 1024          # sequence (after CLS pad)
D = 1024          # model dim
H = 16
DH = 64
NT = N // 128     # 8 row tiles
JC = 640          # compacted key count (kept keys padded)
JT = JC // 128    # 5 compacted j tiles
SCALE_H = DH ** -0.5     # 1/8
SCALE_P = D ** -0.5      # 1/32
NEG = -80.0

_CACHED = {}


def _body(nc, tc, es, t_in, t_out):
    pool = lambda **kw: es.enter_context(tc.tile_pool(**kw))
    xtp = pool(name="xt", bufs=8)
    xcp = pool(name="xc", bufs=8)
    qkp = pool(name="qk", bufs=16)
    kcp = pool(name="kc", bufs=8)
    vpp = pool(name="vp", bufs=JT)
    ptp = pool(name="pt", bufs=3)
    aop = pool(name="ao", bufs=8)
    wqp = pool(name="wq", bufs=3)
    wvp = pool(name="wv", bufs=3)
    wk2p = pool(name="wk2", bufs=3)
    wop = pool(name="wo", bufs=3)
    aonp = pool(name="aon", bufs=4)
    pep = pool(name="pe", bufs=2)
    otp = pool(name="ott", bufs=2)
    onep = pool(name="one", bufs=1)
    smallp = pool(name="small", bufs=8)

    # ---- constants ----
    cmask_t = onep.tile([128, JT], F32, name="cmask_t", tag="cmask_t")
    nc.gpsimd.dma_start(out=cmask_t, in_=t_in["cmaskTc"])
    qkT_t = onep.tile([128, NT], F32, name="qkT_t", tag="qkT_t")
    nc.gpsimd.dma_start(out=qkT_t, in_=t_in["qkeepT"])
    fixl = onep.tile([2, N], BF16, name="fixl", tag="fixl")
    nc.gpsimd.dma_start(out=fixl, in_=t_in["fixl_in"])
    fixr = onep.tile([2, N], BF16, name="fixr", tag="fixr")
    nc.gpsimd.dma_start(out=fixr, in_=t_in["fixr_in"])
    ident = onep.tile([128, 128], BF16, name="ident", tag="ident")
    nc.scalar.dma_start(out=ident, in_=t_in["ident_in"])

    # ---- input loads: xcT on scalar queue (needed first), xT on sync ----
    xcts = []
    for t in range(NT):
        xct = xcp.tile([128, JC], F32R, tag="xc", name=f"xct{t}")
        nc.scalar.dma_start(out=xct, in_=t_in["xcT"][t * 128:(t + 1) * 128, :])
        xcts.append(xct)
    xts = []
    for t in range(NT):
        xt = xtp.tile([128, N], F32R, tag="xt", name=f"xt{t}")
        nc.sync.dma_start(out=xt, in_=t_in["xT"][t * 128:(t + 1) * 128, :])
        xts.append(xt)

    # ---- phase 1a: V tiles (position-major bf16, 64 cols + ones per head) --
    v_tiles = []
    for t in range(JT):
        vt = vpp.tile([128, 65 * H], BF16, tag="v", name=f"v{t}")
        v3 = vt.rearrange("p (h d) -> p h d", d=65)
        nc.vector.memset(v3[:, :, 64:65], 1.0)
        v_tiles.append(vt)

    def phase_v(vpsp):
        for c in range(2):
            pss = []
            for jcb in range(JT):
                ps = vpsp.tile([128, 512], F32, tag="vps", name=f"ps_v{c}{jcb}")
                pss.append(ps)
            for kt in range(NT):
                w = wvp.tile([128, 512], F32R, tag="wv", name="w_v")
                nc.gpsimd.dma_start(
                    out=w,
                    in_=t_in["wv_in"][kt * 128:(kt + 1) * 128,
                                      c * 512:(c + 1) * 512])
                for jcb in range(JT):
                    nc.tensor.matmul(
                        pss[jcb][:],
                        xcts[kt][:, jcb * 128:(jcb + 1) * 128],
                        w[:],
                        start=(kt == 0), stop=(kt == NT - 1),
                    )
            for jcb in range(JT):
                v3 = v_tiles[jcb].rearrange("p (h d) -> p h d", d=65)
                src = pss[jcb].rearrange("p (h d) -> p h d", d=64)
                nc.vector.tensor_copy(v3[:, c * 8:(c + 1) * 8, 0:64], src)

    # ---- phase 1b: compact K projection (f-major kc tiles [128, JC]) ----
    kc_tiles = []

    def phase_kc(kcpsp):
        for kfb in range(NT):
            ps = kcpsp.tile([128, N], F32, tag="kcps", name=f"ps_kc{kfb}")
            for kt in range(NT):
                w = wk2p.tile([128, 128], F32R, tag="wk2", name="w_k2")
                nc.sync.dma_start(
                    out=w,
                    in_=t_in["wqkp"][kt * 128:(kt + 1) * 128,
                                     kfb * 256 + 128:kfb * 256 + 256])
                for c2 in range(2):
                    nc.tensor.matmul(
                        ps[:, c2 * 512:c2 * 512 + 320],
                        w[:],
                        xcts[kt][:, c2 * 320:(c2 + 1) * 320],
                        start=(kt == 0), stop=(kt == NT - 1),
                    )
            kc = kcp.tile([128, JC], F32R, tag="kc", name=f"kc{kfb}")
            nc.vector.tensor_copy(kc[:, 0:320], ps[:, 0:320])
            nc.vector.tensor_copy(kc[:, 320:640], ps[:, 512:832])
            kc_tiles.append(kc)
            if kfb == 0 and "dbg_kc" in t_out:
                nc.gpsimd.dma_start(out=t_out["dbg_kc"], in_=kc)
                nc.gpsimd.dma_start(out=t_out["dbg_v"], in_=v_tiles[0])

    with tc.tile_pool(name="vps", bufs=JT, space="PSUM") as vpsp:
        phase_v(vpsp)
        with tc.tile_pool(name="kcps", bufs=1, space="PSUM") as kcpsp:
            phase_kc(kcpsp)

    # ---- phase 2: pipelined head-pair loop + pooled attention ----
    qk_tiles = [None] * (2 * NT)
    ao_tiles = [None] * NT

    def emit_proj(mmp, fpair):
        pair_ps = [
            mmp.tile([128, N], F32, tag="mm", name=f"ps_qk{fpair}{h}")
            for h in (0, 1)
        ]
        for kt in range(NT):
            w = wqp.tile([128, 256], F32R, tag="wq", name="w_qk")
            nc.sync.dma_start(
                out=w,
                in_=t_in["wqkp"][kt * 128:(kt + 1) * 128,
                                 fpair * 256:(fpair + 1) * 256])
            for half in (0, 1):
                for c in range(2):
                    nc.tensor.matmul(
                        pair_ps[half][:, c * 512:(c + 1) * 512],
                        w[:, half * 128:(half + 1) * 128],
                        xts[kt][:, c * 512:(c + 1) * 512],
                        start=(kt == 0), stop=(kt == NT - 1),
                    )
        for half in (0, 1):
            ft = fpair + half * NT
            qk = qkp.tile([128, N], F32R, tag="qk", name=f"qk{ft}")
            nc.vector.tensor_copy(qk, pair_ps[half])
            qk_tiles[ft] = qk

    def emit_heads(mmp, ppp, trp, hp):
        # P accumulators: 3 it-chunks of 130 cols per bank-sized tile
        P_tiles = [
            ppp.tile([128, 512], F32, tag="ppa", bufs=1, name=f"P{hp}a"),
            ppp.tile([128, 512], F32, tag="ppb", bufs=1, name=f"P{hp}b"),
            ppp.tile([128, 512], F32, tag="ppc", bufs=1, name=f"P{hp}c"),
        ]

        for jt in range(JT):
            pts = []
            for u in (0, 1):
                dp = mmp.tile([128, N], F32, tag="mm", name=f"dp{hp}{u}{jt}")
                off = u * 64
                for c in range(2):
                    nc.tensor.matmul(
                        dp[:, c * 512:(c + 1) * 512],
                        kc_tiles[hp][off:off + 64, jt * 128:(jt + 1) * 128],
                        qk_tiles[hp][off:off + 64, c * 512:(c + 1) * 512],
                        start=True, stop=True,
                    )
                pt = ptp.tile([128, N], BF16, tag="pt", name=f"pt{hp}{u}{jt}")
                nc.scalar.activation(
                    pt, dp, EXP, bias=cmask_t[:, jt:jt + 1], scale=SCALE_H)
                pts.append(pt)
                if hp == 0 and u == 0 and jt == 0 and "dbg_pt" in t_out:
                    nc.gpsimd.dma_start(out=t_out["dbg_pt"], in_=pt)
            for u in (0, 1):
                h = 2 * hp + u
                vh = v_tiles[jt][:, h * 65:(h + 1) * 65]
                for it in range(NT):
                    g, cc = divmod(it, 3)
                    base = cc * 130 + u * 65
                    # start zeroes the whole 2KB PSUM zero-region, so only
                    # the first matmul touching each bank may set it; only
                    # the last one may stop.
                    first = jt == 0 and u == 0 and cc == 0
                    last = (jt == JT - 1 and u == 1
                            and it == min(g * 3 + 2, NT - 1))
                    nc.tensor.matmul(
                        P_tiles[g][:, base:base + 65],
                        pts[u][:, it * 128:(it + 1) * 128],
                        vh,
                        start=first, stop=last,
                    )

        if hp == 0 and "dbg_P" in t_out:
            dbgp = smallp.tile([128, 512], F32, tag="dbgp", bufs=1, name="dbgp")
            nc.vector.tensor_copy(dbgp, P_tiles[0])
            nc.gpsimd.dma_start(out=t_out["dbg_P"], in_=dbgp)

        # normalize + transpose to aoT[hp]
        ao = aop.tile([128, N], BF16, tag="ao", name=f"ao{hp}")
        ao_tiles[hp] = ao
        for half in (0, 1):
            tr = trp.tile([128, 512], BF16, tag="tr", name=f"tr{hp}{half}")
            for q4 in range(4):
                it = half * 4 + q4
                g, cc = divmod(it, 3)
                rec = smallp.tile([128, 2], F32, tag="rec", bufs=2,
                                  name=f"rec{hp}{it}")
                nc.vector.reciprocal(
                    rec, P_tiles[g][:, cc * 130 + 64:cc * 130 + 130:65])
                rq = smallp.tile([128, 2], F32, tag="rq", bufs=2,
                                 name=f"rq{hp}{it}")
                nc.vector.tensor_scalar_mul(rq, rec, qkT_t[:, it:it + 1])
                aon = aonp.tile([128, 128], BF16, tag="aon",
                                name=f"aon{hp}{it}")
                for u in (0, 1):
                    base = cc * 130 + u * 65
                    nc.vector.tensor_scalar_mul(
                        aon[:, u * 64:(u + 1) * 64],
                        P_tiles[g][:, base:base + 64],
                        rq[:, u:u + 1])
                nc.tensor.matmul(
                    tr[:, q4 * 128:(q4 + 1) * 128],
                    aon[:], ident[:], is_transpose=True)
            nc.vector.tensor_copy(ao[:, half * 512:(half + 1) * 512], tr)
        if hp == 0 and "dbg_ao" in t_out:
            nc.gpsimd.dma_start(out=t_out["dbg_ao"], in_=ao)

    def emit_pooled(mmp, it):
        ps = mmp.tile([128, N], F32, tag="mm", name=f"ps_pool{it}")
        for c in range(2):
            for ft in range(NT):
                nc.tensor.matmul(
                    ps[:, c * 512:(c + 1) * 512],
                    qk_tiles[ft][:, it * 128:(it + 1) * 128],
                    qk_tiles[NT + ft][:, c * 512:(c + 1) * 512],
                    start=(ft == 0), stop=(ft == NT - 1),
                )
        pe = pep.tile([128, N], F32, tag="pe", name=f"pe{it}")
        sums = smallp.tile([128, 1], F32, tag="sm", name=f"sums{it}")
        nc.scalar.activation(pe, ps, EXP, scale=SCALE_P, accum_out=sums)
        rec = smallp.tile([128, 1], F32, tag="rc", name=f"rcp{it}")
        nc.vector.reciprocal(rec, sums)
        nc.vector.tensor_scalar_mul(pe, pe, rec)
        nc.gpsimd.dma_start(out=t_out["attn"][it * 128:(it + 1) * 128, :],
                            in_=pe)

    with (
        tc.tile_pool(name="mm", bufs=2, space="PSUM") as mmp,
        tc.tile_pool(name="pp", bufs=1, space="PSUM") as ppp,
        tc.tile_pool(name="tr", bufs=1, space="PSUM") as trp,
    ):
        emit_proj(mmp, 0)
        for hp in range(NT):
            if hp + 1 < NT:
                emit_proj(mmp, hp + 1)
            emit_heads(mmp, ppp, trp, hp)
        for it in range(NT):
            emit_pooled(mmp, it)

    # ---- phase 3: out projection (d-split passes, K=2 rank-2 fix) ----
    def phase_out(opsp):
        for dh in range(2):
            pss = [
                opsp.tile([128, 512], F32, tag="ops", name=f"ps_o{dh}{i}")
                for i in range(NT)
            ]
            for ft in range(NT):
                w = wop.tile([128, 512], BF16, tag="wo", name="w_o")
                nc.scalar.dma_start(
                    out=w,
                    in_=t_in["wout"][ft * 128:(ft + 1) * 128,
                                     dh * 512:(dh + 1) * 512])
                for it in range(NT):
                    nc.tensor.matmul(
                        pss[it][:],
                        ao_tiles[ft][:, it * 128:(it + 1) * 128],
                        w[:],
                        start=(ft == 0), stop=False,
                    )
            for it in range(NT):
                nc.tensor.matmul(
                    pss[it][:],
                    fixl[:, it * 128:(it + 1) * 128],
                    fixr[:, dh * 512:(dh + 1) * 512],
                    start=False, stop=True,
                )
                ot = otp.tile([128, 512], F32, tag="ot", name=f"ot{dh}{it}")
                nc.vector.tensor_copy(ot, pss[it])
                eng = nc.sync if dh == 0 else nc.scalar
                eng.dma_start(
                    out=t_out["out"][it * 128:(it + 1) * 128,
                                     dh * 512:(dh + 1) * 512],
                    in_=ot)

    with tc.tile_pool(name="ops", bufs=8, space="PSUM") as opsp:
        phase_out(opsp)


def build_nc():
    nc = bacc.Bacc("TRN2", target_bir_lowering=False, debug=False,
                   num_devices=8)
    t_in = {
        "xT": nc.dram_tensor("xT", [D, N], F32R, kind="ExternalInput").ap(),
        "xcT": nc.dram_tensor("xcT", [D, JC], F32R, kind="ExternalInput").ap(),
        "wqkp": nc.dram_tensor("wqkp", [D, 2 * D], F32R,
                               kind="ExternalInput").ap(),
        "wv_in": nc.dram_tensor("wv_in", [D, D], F32R,
                                kind="ExternalInput").ap(),
        "wout": nc.dram_tensor("wout", [D, D], BF16,
                               kind="ExternalInput").ap(),
        "cmaskTc": nc.dram_tensor("cmaskTc", [128, JT], F32,
                                  kind="ExternalInput").ap(),
        "qkeepT": nc.dram_tensor("qkeepT", [128, NT], F32,
                                 kind="ExternalInput").ap(),
        "fixl_in": nc.dram_tensor("fixl_in", [2, N], BF16,
                                  kind="ExternalInput").ap(),
        "fixr_in": nc.dram_tensor("fixr_in", [2, N], BF16,
                                  kind="ExternalInput").ap(),
        "ident_in": nc.dram_tensor("ident_in", [128, 128], BF16,
                                   kind="ExternalInput").ap(),
    }
    t_out = {
        "out": nc.dram_tensor("out", [N, D], F32, kind="ExternalOutput").ap(),
        "attn": nc.dram_tensor("attn", [N, N], F32,
                               kind="ExternalOutput").ap(),
    }
    if os.environ.get("ATTN_DEBUG"):
        t_out["dbg_kc"] = nc.dram_tensor(
            "dbg_kc", [128, JC], F32R, kind="ExternalOutput").ap()
        t_out["dbg_v"] = nc.dram_tensor(
            "dbg_v", [128, 65 * H], BF16, kind="ExternalOutput").ap()
        t_out["dbg_pt"] = nc.dram_tensor(
            "dbg_pt", [128, N], BF16, kind="ExternalOutput").ap()
        t_out["dbg_P"] = nc.dram_tensor(
            "dbg_P", [128, 512], F32, kind="ExternalOutput").ap()
        t_out["dbg_ao"] = nc.dram_tensor(
            "dbg_ao", [128, N], BF16, kind="ExternalOutput").ap()
    with tile.TileContext(
            nc, trace_sim=bool(os.environ.get('ATTN_TRACE_SIM'))) as tc:
        with ExitStack() as es:
            _body(nc, tc, es, t_in, t_out)
    nc.compile()
    return nc


def _bf16(a):
    import ml_dtypes
    return np.asarray(a, np.float32).astype(ml_dtypes.bfloat16)


def _host_prep(x, mask, w_qkv, w_out, b_out):
    wq3 = w_qkv.reshape(D, 3, NT, 128)
    wqkp = np.ascontiguousarray(
        np.stack([wq3[:, 0], wq3[:, 1]], axis=2).reshape(D, 2 * D))
    wv = np.ascontiguousarray(w_qkv[:, 2 * D:])
    wout_b = _bf16(w_out)
    ident = _bf16(np.eye(128, dtype=np.float32))
    in_maps = []
    for b in range(B):
        m = np.concatenate([[True], mask[b]])               # [N]
        keep = np.nonzero(m)[0]
        nk = len(keep)
        assert nk <= JC, f"keep count {nk} exceeds JC={JC}"
        idx = np.concatenate([keep, np.zeros(JC - nk, np.int64)])
        cm_c = np.where(np.arange(JC) < nk, 0.0, NEG).astype(np.float32)
        xTb = np.ascontiguousarray(x[b].T)                  # [D, N]
        xcT = np.ascontiguousarray(xTb[:, idx])             # [D, JC]
        mf = m.astype(np.float32)
        ybar = (x[b].mean(axis=0) @ wv) @ w_out             # [D]
        in_maps.append({
            "xT": xTb,
            "xcT": xcT,
            "wqkp": wqkp,
            "wv_in": wv,
            "wout": wout_b,
            "cmaskTc": np.ascontiguousarray(cm_c.reshape(JT, 128).T),
            "qkeepT": np.ascontiguousarray(mf.reshape(NT, 128).T),
            "fixl_in": _bf16(np.stack([np.ones(N, np.float32), 1.0 - mf])),
            "fixr_in": _bf16(np.stack([b_out, ybar])),
            "ident_in": ident,
        })
    return in_maps


def kernel(x, mask, w_qkv, w_out, b_out, **run_kw):
    if "nc" not in _CACHED:
        _CACHED["nc"] = build_nc()
    nc = _CACHED["nc"]
    in_maps = _host_prep(
        np.asarray(x, np.float32), np.asarray(mask),
        np.asarray(w_qkv, np.float32), np.asarray(w_out, np.float32),
        np.asarray(b_out, np.float32))
    try:
        res = bass_utils.run_bass_kernel_spmd(
            nc, in_maps, core_ids=list(range(B)), **run_kw)
    except Exception:
        # transient NRT device wedge: retry once
        res = bass_utils.run_bass_kernel_spmd(
            nc, in_maps, core_ids=list(range(B)), **run_kw)
    out = np.stack([res.results[b]["out"] for b in range(B)])
    attn_ = np.stack([res.results[b]["attn"] for b in range(B)])
    _CACHED["last_results"] = res
    return out, attn_


# revision 17
# speedup vs baseline: 1.4096x; 1.0838x over previous
"""Fused attention block (qkv proj + pooled attention + 16-head masked
attention + out proj) for TRN2, batch-parallel across 8 NeuronCores.

Key optimizations vs the straightforward version:
  - Key/value compaction: masked keys (~50%) are dropped on the host by
    gathering kept columns of x^T into xcT [D, 640] (padded with CLS,
    pad slots get a -80 exp bias). V-proj, head dots, exp, and attn@V
    all shrink by ~3/8. The pooled attention is unmasked and keeps the
    full K.
  - attn@V is "flipped": stationary = exp(dots) [jc, i-block], moving =
    per-head V plus a ones column [jc, 65] in bf16 (bf16 keeps 1
    cycle/row at a 65-wide moving operand). Output lands i-major with
    the softmax denominator in the 65th column -> per-partition
    normalization on DVE, then a PE transpose to feature-major aoT.
  - Masked queries are zeroed via a per-partition qkeep multiply and
    fixed at the out-projection by a single K=2 rank-2 correction
    matmul (ones x bout + (1-m)/N x ybar), ybar computed on the host.
  - Software pipelining: QK-proj matmuls of pair F+1 are interleaved
    into the ACT-paced dots/exp stretch of pair F (separate PSUM pools
    so slot reuse cannot serialize them); attn@V of pair F runs on the
    resident exp tiles right after. Transposes are deferred into the
    pooled-attention phase; out-projection is d-split with one bf16
    wout stream.
  - PSUM start-bit semantics: `start` zeroes a whole 2KB bank region,
    so per bank only the first matmul sets it.
"""
import os
import sys
from contextlib import ExitStack

sys.path.insert(0, "/opt/trn_rl_repo")

import numpy as np

import concourse.bass as bass
import concourse.mybir as mybir
import concourse.tile as tile
from concourse import bacc, bass_utils

F32 = mybir.dt.float32
F32R = mybir.dt.float32r
BF16 = mybir.dt.bfloat16
EXP = mybir.ActivationFunctionType.Exp

B = 8
N = 1024          # sequence (after CLS pad)
D = 1024          # model dim
H = 16
DH = 64
NT = N // 128     # 8 row tiles
JC = 640          # compacted key count (kept keys padded)
JT = JC // 128    # 5 compacted j tiles
SCALE_H = DH ** -0.5     # 1/8
SCALE_P = D ** -0.5      # 1/32
NEG = -80.0

_CACHED = {}


def _body(nc, tc, es, t_in, t_out):
    pool = lambda **kw: es.enter_context(tc.tile_pool(**kw))
    xtp = pool(name="xt", bufs=8)
    qkp = pool(name="qk", bufs=16)
    kcp = pool(name="kc", bufs=8)
    vpp = pool(name="vp", bufs=JT)
    aop = pool(name="ao", bufs=8)
    aonp = pool(name="aon", bufs=64)
    onep = pool(name="one", bufs=1)
    smallp = pool(name="small", bufs=8)

    # ---- constants ----
    cmask_t = onep.tile([128, JT], F32, name="cmask_t", tag="cmask_t")
    nc.gpsimd.dma_start(out=cmask_t, in_=t_in["cmaskTc"])
    qkT_t = onep.tile([128, NT], F32, name="qkT_t", tag="qkT_t")
    nc.gpsimd.dma_start(out=qkT_t, in_=t_in["qkeepT"])
    fixl = onep.tile([2, N], BF16, name="fixl", tag="fixl")
    nc.gpsimd.dma_start(out=fixl, in_=t_in["fixl_in"])
    fixr = onep.tile([2, N], BF16, name="fixr", tag="fixr")
    nc.gpsimd.dma_start(out=fixr, in_=t_in["fixr_in"])
    ident = onep.tile([128, 128], BF16, name="ident", tag="ident")
    nc.scalar.dma_start(out=ident, in_=t_in["ident_in"])

    xts = []
    for t in range(NT):
        xt = xtp.tile([128, N], F32R, tag="xt", name=f"xt{t}")
        nc.sync.dma_start(out=xt, in_=t_in["xT"][t * 128:(t + 1) * 128, :])
        xts.append(xt)

    v_tiles = []
    for t in range(JT):
        vt = vpp.tile([128, 65 * H], BF16, tag="v", name=f"v{t}")
        v3 = vt.rearrange("p (h d) -> p h d", d=65)
        nc.vector.memset(v3[:, :, 64:65], 1.0)
        v_tiles.append(vt)

    kc_tiles = []
    qk_tiles = [None] * (2 * NT)
    ao_tiles = [None] * NT
    aon_tiles = [[None] * NT for _ in range(NT)]

    def phase_vkc(xcp, wvp, wk2p, vpsp, kcpsp):
        xcts = []
        for t in range(NT):
            xct = xcp.tile([128, JC], F32R, tag="xc", name=f"xct{t}")
            nc.scalar.dma_start(out=xct,
                                in_=t_in["xcT"][t * 128:(t + 1) * 128, :])
            xcts.append(xct)
        # V projection (position-major, bf16 + ones col)
        for c in range(2):
            pss = [vpsp.tile([128, 512], F32, tag="vps", name=f"ps_v{c}{j}")
                   for j in range(JT)]
            for kt in range(NT):
                w = wvp.tile([128, 512], F32R, tag="wv", name="w_v")
                nc.gpsimd.dma_start(
                    out=w, in_=t_in["wv_in"][kt * 128:(kt + 1) * 128,
                                             c * 512:(c + 1) * 512])
                for jcb in range(JT):
                    nc.tensor.matmul(
                        pss[jcb][:],
                        xcts[kt][:, jcb * 128:(jcb + 1) * 128],
                        w[:],
                        start=(kt == 0), stop=(kt == NT - 1),
                    )
            for jcb in range(JT):
                v3 = v_tiles[jcb].rearrange("p (h d) -> p h d", d=65)
                src = pss[jcb].rearrange("p (h d) -> p h d", d=64)
                nc.vector.tensor_copy(v3[:, c * 8:(c + 1) * 8, 0:64], src)
        # compact K projection (f-major kc tiles [128, JC])
        for kfb in range(NT):
            ps = kcpsp.tile([128, N], F32, tag="kcps", name=f"ps_kc{kfb}")
            for kt in range(NT):
                w = wk2p.tile([128, 128], F32R, tag="wk2", name="w_k2")
                nc.sync.dma_start(
                    out=w, in_=t_in["wqkp"][kt * 128:(kt + 1) * 128,
                                            kfb * 256 + 128:kfb * 256 + 256])
                for c2 in range(2):
                    nc.tensor.matmul(
                        ps[:, c2 * 512:c2 * 512 + 320],
                        w[:],
                        xcts[kt][:, c2 * 320:(c2 + 1) * 320],
                        start=(kt == 0), stop=(kt == NT - 1),
                    )
            kc = kcp.tile([128, JC], F32R, tag="kc", name=f"kc{kfb}")
            nc.vector.tensor_copy(kc[:, 0:320], ps[:, 0:320])
            nc.vector.tensor_copy(kc[:, 320:640], ps[:, 512:832])
            kc_tiles.append(kc)
            if kfb == 0 and "dbg_kc" in t_out:
                nc.gpsimd.dma_start(out=t_out["dbg_kc"], in_=kc)
                nc.gpsimd.dma_start(out=t_out["dbg_v"], in_=v_tiles[0])

    def head_loop(ptp, wqp, projp, dpp, Ppp):
        def load_w(fpair):
            ws = []
            for kt in range(NT):
                w = wqp.tile([128, 256], F32R, tag="wq", name=f"wq{fpair}{kt}")
                nc.sync.dma_start(
                    out=w, in_=t_in["wqkp"][kt * 128:(kt + 1) * 128,
                                            fpair * 256:(fpair + 1) * 256])
                ws.append(w)
            return ws

        def proj_ops(fpair, ws):
            """Flat op list: 4 sequential [128,512] chunk accumulations."""
            ops = []
            for half in (0, 1):
                ft = fpair + half * NT
                qk_tiles[ft] = qkp.tile([128, N], F32R, tag="qk",
                                        name=f"qk{ft}")
            for half in (0, 1):
                ft = fpair + half * NT
                for c in (0, 1):
                    ps = projp.tile([128, 512], F32, tag="pj",
                                    name=f"pj{ft}{c}")
                    for kt in range(NT):
                        ops.append(lambda ps=ps, kt=kt, half=half, c=c:
                                   nc.tensor.matmul(
                                       ps[:],
                                       ws[kt][:, half * 128:(half + 1) * 128],
                                       xts[kt][:, c * 512:(c + 1) * 512],
                                       start=(kt == 0), stop=(kt == NT - 1)))
                    ops.append(lambda ps=ps, ft=ft, c=c:
                               nc.vector.tensor_copy(
                                   qk_tiles[ft][:, c * 512:(c + 1) * 512], ps))
            return ops

        ws = load_w(0)
        pend = proj_ops(0, ws)
        while pend:
            pend.pop(0)()

        for hp in range(NT):
            if hp + 1 < NT:
                ws = load_w(hp + 1)
                pend = proj_ops(hp + 1, ws)
            else:
                pend = []

            # dots + exp stretch, proj(F+1) matmuls woven in
            pts = {}
            for s in range(2 * JT):
                jt, u = divmod(s, 2)
                dp = dpp.tile([128, N], F32, tag="dp", name=f"dp{hp}{u}{jt}")
                off = u * 64
                for c in range(2):
                    nc.tensor.matmul(
                        dp[:, c * 512:(c + 1) * 512],
                        kc_tiles[hp][off:off + 64, jt * 128:(jt + 1) * 128],
                        qk_tiles[hp][off:off + 64, c * 512:(c + 1) * 512],
                        start=True, stop=True,
                    )
                pt = ptp.tile([128, N], BF16, tag="pt", name=f"pt{hp}{u}{jt}")
                nc.scalar.activation(
                    pt, dp, EXP, bias=cmask_t[:, jt:jt + 1], scale=SCALE_H)
                pts[(u, jt)] = pt
                if hp == 0 and s == 0 and "dbg_pt" in t_out:
                    nc.gpsimd.dma_start(out=t_out["dbg_pt"], in_=pt)
                for _ in range(4):
                    if pend:
                        pend.pop(0)()
            while pend:
                pend.pop(0)()

            # attn@V on resident pt tiles + per-partition normalization
            for it in range(NT):
                P = Ppp.tile([128, 512], F32, tag="P", name=f"P{hp}{it}")
                for jt in range(JT):
                    for u in (0, 1):
                        h = 2 * hp + u
                        nc.tensor.matmul(
                            P[:, u * 65:u * 65 + 65],
                            pts[(u, jt)][:, it * 128:(it + 1) * 128],
                            v_tiles[jt][:, h * 65:(h + 1) * 65],
                            start=(jt == 0 and u == 0),
                            stop=(jt == JT - 1 and u == 1),
                        )
                if hp == 0 and it == 0 and "dbg_P" in t_out:
                    dbgp = smallp.tile([128, 512], F32, tag="dbgp", bufs=1,
                                       name="dbgp")
                    nc.vector.tensor_copy(dbgp, P)
                    nc.gpsimd.dma_start(out=t_out["dbg_P"], in_=dbgp)
                rec = smallp.tile([128, 2], F32, tag="rec", bufs=3,
                                  name=f"rec{hp}{it}")
                nc.vector.reciprocal(rec, P[:, 64:130:65])
                rq = smallp.tile([128, 2], F32, tag="rq", bufs=3,
                                 name=f"rq{hp}{it}")
                nc.vector.tensor_scalar_mul(rq, rec, qkT_t[:, it:it + 1])
                aon = aonp.tile([128, 128], BF16, tag="aon",
                                name=f"aon{hp}{it}")
                for u in (0, 1):
                    nc.vector.tensor_scalar_mul(
                        aon[:, u * 64:(u + 1) * 64],
                        P[:, u * 65:u * 65 + 64],
                        rq[:, u:u + 1])
                aon_tiles[hp][it] = aon

    def phase_pooled(pep, pop, trp):
        for it in range(NT):
            ps = pop.tile([128, N], F32, tag="po", name=f"ps_pool{it}")
            for c in range(2):
                for ft in range(NT):
                    nc.tensor.matmul(
                        ps[:, c * 512:(c + 1) * 512],
                        qk_tiles[ft][:, it * 128:(it + 1) * 128],
                        qk_tiles[NT + ft][:, c * 512:(c + 1) * 512],
                        start=(ft == 0), stop=(ft == NT - 1),
                    )
            pe = pep.tile([128, N], F32, tag="pe", name=f"pe{it}")
            sums = smallp.tile([128, 1], F32, tag="sm", name=f"sums{it}")
            nc.scalar.activation(pe, ps, EXP, scale=SCALE_P, accum_out=sums)
            rec = smallp.tile([128, 1], F32, tag="rc", name=f"rcp{it}")
            nc.vector.reciprocal(rec, sums)
            nc.vector.tensor_scalar_mul(pe, pe, rec)
            nc.gpsimd.dma_start(out=t_out["attn"][it * 128:(it + 1) * 128, :],
                                in_=pe)
            # transpose group hp=it of aon -> aoT (fills PE gaps here)
            hp = it
            tr = trp.tile([128, N], BF16, tag="tr", name=f"tr{hp}")
            for k in range(NT):
                nc.tensor.matmul(
                    tr[:, k * 128:(k + 1) * 128],
                    aon_tiles[hp][k][:], ident[:], is_transpose=True)
            ao = aop.tile([128, N], BF16, tag="ao", name=f"ao{hp}")
            nc.vector.tensor_copy(ao, tr)
            ao_tiles[hp] = ao
            if hp == 0 and "dbg_ao" in t_out:
                nc.gpsimd.dma_start(out=t_out["dbg_ao"], in_=ao)

    def phase_out(wop, otp, opsp):
        for dh in range(2):
            pss = [opsp.tile([128, 512], F32, tag="ops", name=f"ps_o{dh}{i}")
                   for i in range(NT)]
            for ft in range(NT):
                w = wop.tile([128, 512], BF16, tag="wo", name="w_o")
                nc.scalar.dma_start(
                    out=w, in_=t_in["wout"][ft * 128:(ft + 1) * 128,
                                            dh * 512:(dh + 1) * 512])
                for it in range(NT):
                    nc.tensor.matmul(
                        pss[it][:],
                        ao_tiles[ft][:, it * 128:(it + 1) * 128],
                        w[:],
                        start=(ft == 0), stop=False,
                    )
            for it in range(NT):
                nc.tensor.matmul(
                    pss[it][:],
                    fixl[:, it * 128:(it + 1) * 128],
                    fixr[:, dh * 512:(dh + 1) * 512],
                    start=False, stop=True,
                )
                ot = otp.tile([128, 512], F32, tag="ot", bufs=4,
                              name=f"ot{dh}{it}")
                nc.vector.tensor_copy(ot, pss[it])
                eng = nc.sync if it % 2 == 0 else nc.scalar
                eng.dma_start(
                    out=t_out["out"][it * 128:(it + 1) * 128,
                                     dh * 512:(dh + 1) * 512],
                    in_=ot)

    with (
        tc.tile_pool(name="xc", bufs=8) as xcp,
        tc.tile_pool(name="wv", bufs=3) as wvp,
        tc.tile_pool(name="wk2", bufs=3) as wk2p,
        tc.tile_pool(name="vps", bufs=JT, space="PSUM") as vpsp,
        tc.tile_pool(name="kcps", bufs=1, space="PSUM") as kcpsp,
    ):
        phase_vkc(xcp, wvp, wk2p, vpsp, kcpsp)

    with (
        tc.tile_pool(name="pt", bufs=12) as ptp,
        tc.tile_pool(name="wq", bufs=10) as wqp,
        tc.tile_pool(name="pj", bufs=2, space="PSUM") as projp,
        tc.tile_pool(name="dp", bufs=2, space="PSUM") as dpp,
        tc.tile_pool(name="Pp", bufs=2, space="PSUM") as Ppp,
    ):
        head_loop(ptp, wqp, projp, dpp, Ppp)

    with (
        tc.tile_pool(name="pe", bufs=2) as pep,
        tc.tile_pool(name="po", bufs=2, space="PSUM") as pop,
        tc.tile_pool(name="tr", bufs=2, space="PSUM") as trp,
    ):
        phase_pooled(pep, pop, trp)

    with (
        tc.tile_pool(name="wo", bufs=3) as wop,
        tc.tile_pool(name="ott", bufs=4) as otp,
        tc.tile_pool(name="ops", bufs=8, space="PSUM") as opsp,
    ):
        phase_out(wop, otp, opsp)


def build_nc():
    nc = bacc.Bacc("TRN2", target_bir_lowering=False, debug=False,
                   num_devices=8)
    t_in = {
        "xT": nc.dram_tensor("xT", [D, N], F32R, kind="ExternalInput").ap(),
        "xcT": nc.dram_tensor("xcT", [D, JC], F32R, kind="ExternalInput").ap(),
        "wqkp": nc.dram_tensor("wqkp", [D, 2 * D], F32R,
                               kind="ExternalInput").ap(),
        "wv_in": nc.dram_tensor("wv_in", [D, D], F32R,
                                kind="ExternalInput").ap(),
        "wout": nc.dram_tensor("wout", [D, D], BF16,
                               kind="ExternalInput").ap(),
        "cmaskTc": nc.dram_tensor("cmaskTc", [128, JT], F32,
                                  kind="ExternalInput").ap(),
        "qkeepT": nc.dram_tensor("qkeepT", [128, NT], F32,
                                 kind="ExternalInput").ap(),
        "fixl_in": nc.dram_tensor("fixl_in", [2, N], BF16,
                                  kind="ExternalInput").ap(),
        "fixr_in": nc.dram_tensor("fixr_in", [2, N], BF16,
                                  kind="ExternalInput").ap(),
        "ident_in": nc.dram_tensor("ident_in", [128, 128], BF16,
                                   kind="ExternalInput").ap(),
    }
    t_out = {
        "out": nc.dram_tensor("out", [N, D], F32, kind="ExternalOutput").ap(),
        "attn": nc.dram_tensor("attn", [N, N], F32,
                               kind="ExternalOutput").ap(),
    }
    if os.environ.get("ATTN_DEBUG"):
        t_out["dbg_kc"] = nc.dram_tensor(
            "dbg_kc", [128, JC], F32R, kind="ExternalOutput").ap()
        t_out["dbg_v"] = nc.dram_tensor(
            "dbg_v", [128, 65 * H], BF16, kind="ExternalOutput").ap()
        t_out["dbg_pt"] = nc.dram_tensor(
            "dbg_pt", [128, N], BF16, kind="ExternalOutput").ap()
        t_out["dbg_P"] = nc.dram_tensor(
            "dbg_P", [128, 512], F32, kind="ExternalOutput").ap()
        t_out["dbg_ao"] = nc.dram_tensor(
            "dbg_ao", [128, N], BF16, kind="ExternalOutput").ap()
    with tile.TileContext(
            nc, trace_sim=bool(os.environ.get('ATTN_TRACE_SIM'))) as tc:
        with ExitStack() as es:
            _body(nc, tc, es, t_in, t_out)
    nc.compile()
    return nc


def _bf16(a):
    import ml_dtypes
    return np.asarray(a, np.float32).astype(ml_dtypes.bfloat16)


def _host_prep(x, mask, w_qkv, w_out, b_out):
    wq3 = w_qkv.reshape(D, 3, NT, 128)
    wqkp = np.ascontiguousarray(
        np.stack([wq3[:, 0], wq3[:, 1]], axis=2).reshape(D, 2 * D))
    wv = np.ascontiguousarray(w_qkv[:, 2 * D:])
    wout_b = _bf16(w_out)
    ident = _bf16(np.eye(128, dtype=np.float32))
    in_maps = []
    for b in range(B):
        m = np.concatenate([[True], mask[b]])               # [N]
        keep = np.nonzero(m)[0]
        nk = len(keep)
        assert nk <= JC, f"keep count {nk} exceeds JC={JC}"
        idx = np.concatenate([keep, np.zeros(JC - nk, np.int64)])
        cm_c = np.where(np.arange(JC) < nk, 0.0, NEG).astype(np.float32)
        xTb = np.ascontiguousarray(x[b].T)                  # [D, N]
        xcT = np.ascontiguousarray(xTb[:, idx])             # [D, JC]
        mf = m.astype(np.float32)
        ybar = (x[b].mean(axis=0) @ wv) @ w_out             # [D]
        in_maps.append({
            "xT": xTb,
            "xcT": xcT,
            "wqkp": wqkp,
            "wv_in": wv,
            "wout": wout_b,
            "cmaskTc": np.ascontiguousarray(cm_c.reshape(JT, 128).T),
            "qkeepT": np.ascontiguousarray(mf.reshape(NT, 128).T),
            "fixl_in": _bf16(np.stack([np.ones(N, np.float32), 1.0 - mf])),
            "fixr_in": _bf16(np.stack([b_out, ybar])),
            "ident_in": ident,
        })
    return in_maps


def kernel(x, mask, w_qkv, w_out, b_out, **run_kw):
    if "nc" not in _CACHED:
        _CACHED["nc"] = build_nc()
    nc = _CACHED["nc"]
    in_maps = _host_prep(
        np.asarray(x, np.float32), np.asarray(mask),
        np.asarray(w_qkv, np.float32), np.asarray(w_out, np.float32),
        np.asarray(b_out, np.float32))
    try:
        res = bass_utils.run_bass_kernel_spmd(
            nc, in_maps, core_ids=list(range(B)), **run_kw)
    except Exception:
        # transient NRT device wedge: retry once
        res = bass_utils.run_bass_kernel_spmd(
            nc, in_maps, core_ids=list(range(B)), **run_kw)
    out = np.stack([res.results[b]["out"] for b in range(B)])
    attn_ = np.stack([res.results[b]["attn"] for b in range(B)])
    _CACHED["last_results"] = res
    return out, attn_


# revision 32
# speedup vs baseline: 1.6175x; 1.1475x over previous
"""Fused attention block (qkv proj + pooled attention + 16-head masked
attention + out proj) for TRN2, batch-parallel across 8 NeuronCores.

Key optimizations vs the straightforward version:
  - Key/value compaction: masked keys (~50%) are dropped on the host by
    gathering kept columns of x^T into xcT [D, 640] (padded with CLS,
    pad slots get a -80 exp bias). V-proj, head dots, exp, and attn@V
    all shrink by ~3/8. The pooled attention is unmasked and keeps the
    full K.
  - attn@V is "flipped": stationary = exp(dots) [jc, i-block], moving =
    per-head V plus a ones column [jc, 65] in bf16 (bf16 keeps 1
    cycle/row at a 65-wide moving operand). Output lands i-major with
    the softmax denominator in the 65th column -> per-partition
    normalization on DVE, then a PE transpose to feature-major aoT.
  - Masked queries are zeroed via a per-partition qkeep multiply and
    fixed at the out-projection by a single K=2 rank-2 correction
    matmul (ones x bout + (1-m)/N x ybar), ybar computed on the host.
  - Software pipelining: QK-proj matmuls of pair F+1 are interleaved
    into the ACT-paced dots/exp stretch of pair F (separate PSUM pools
    so slot reuse cannot serialize them); attn@V of pair F runs on the
    resident exp tiles right after. Transposes are deferred into the
    pooled-attention phase; out-projection is d-split with one bf16
    wout stream.
  - PSUM start-bit semantics: `start` zeroes a whole 2KB bank region,
    so per bank only the first matmul sets it.
"""
import os
import sys
from contextlib import ExitStack

sys.path.insert(0, "/opt/trn_rl_repo")

import numpy as np

import concourse.bass as bass
import concourse.mybir as mybir
import concourse.tile as tile
from concourse import bacc, bass_utils

F32 = mybir.dt.float32
F32R = mybir.dt.float32r
BF16 = mybir.dt.bfloat16
EXP = mybir.ActivationFunctionType.Exp

B = 8
N = 1024          # sequence (after CLS pad)
D = 1024          # model dim
H = 16
DH = 64
NT = N // 128     # 8 row tiles
JC = 640          # compacted key count (kept keys padded)
JT = JC // 128    # 5 compacted j tiles
SCALE_H = DH ** -0.5     # 1/8
SCALE_P = D ** -0.5      # 1/32
NEG = -80.0

_CACHED = {}


def _body(nc, tc, es, t_in, t_out):
    pool = lambda **kw: es.enter_context(tc.tile_pool(**kw))
    xtp = pool(name="xt", bufs=8)
    qkp = pool(name="qk", bufs=16)
    kcp = pool(name="kc", bufs=8)
    vpp = pool(name="vp", bufs=JT)
    aonp = pool(name="aon", bufs=64)
    onep = pool(name="one", bufs=1)
    smallp = pool(name="small", bufs=8)

    # ---- constants ----
    cmask_t = onep.tile([128, JT], F32, name="cmask_t", tag="cmask_t")
    nc.gpsimd.dma_start(out=cmask_t, in_=t_in["cmaskTc"])
    qkT_t = onep.tile([128, NT], F32, name="qkT_t", tag="qkT_t")
    nc.gpsimd.dma_start(out=qkT_t, in_=t_in["qkeepT"])
    fixl = onep.tile([2, N], BF16, name="fixl", tag="fixl")
    nc.gpsimd.dma_start(out=fixl, in_=t_in["fixl_in"])
    fixr = onep.tile([2, N], BF16, name="fixr", tag="fixr")
    nc.gpsimd.dma_start(out=fixr, in_=t_in["fixr_in"])
    ident = onep.tile([128, 128], BF16, name="ident", tag="ident")
    nc.gpsimd.dma_start(out=ident, in_=t_in["ident_in"])

    xts = []

    v_tiles = []
    for t in range(JT):
        vt = vpp.tile([128, 65 * H], BF16, tag="v", name=f"v{t}")
        v3 = vt.rearrange("p (h d) -> p h d", d=65)
        nc.vector.memset(v3[:, :, 64:65], 1.0)
        v_tiles.append(vt)

    kc_tiles = []
    qk_tiles = [None] * (2 * NT)
    ao_tiles = [None] * NT
    aon_tiles = [[None] * NT for _ in range(NT)]

    def phase_vkc(xcp, wvp, wk2p, vpsp, kcpsp):
        # xcT (then xT) on the Activation queue; batched weights on SP.
        xcts = []
        for t in range(NT):
            xct = xcp.tile([128, JC], F32R, tag="xc", name=f"xct{t}")
            nc.scalar.dma_start(out=xct,
                                in_=t_in["xcT"][t * 128:(t + 1) * 128, :])
            xcts.append(xct)
        for t in range(NT):
            xt = xtp.tile([128, N], F32R, tag="xt", name=f"xt{t}")
            nc.scalar.dma_start(out=xt, in_=t_in["xT"][t * 128:(t + 1) * 128, :])
            xts.append(xt)
        # wv batched: [128, 4*512] tiles, 4 kt row-blocks per load
        wv4 = t_in["wv_in"].rearrange("(kt p) d -> p kt d", p=128)
        wvts = {}
        for c in range(2):
            for g in range(2):
                w = wvp.tile([128, 4, 512], F32R, tag="wv", name=f"w_v{c}{g}")
                nc.sync.dma_start(
                    out=w, in_=wv4[:, g * 4:(g + 1) * 4,
                                   c * 512:(c + 1) * 512])
                wvts[c * 2 + g] = w
        # V projection (position-major, bf16 + ones col)
        for c in range(2):
            pss = [vpsp.tile([128, 512], F32, tag="vps", name=f"ps_v{c}{j}")
                   for j in range(JT)]
            for kt in range(NT):
                w = wvts[c * 2 + kt // 4][:, kt % 4, :]
                for jcb in range(JT):
                    nc.tensor.matmul(
                        pss[jcb][:],
                        xcts[kt][:, jcb * 128:(jcb + 1) * 128],
                        w,
                        start=(kt == 0), stop=(kt == NT - 1),
                    )
            for jcb in range(JT):
                v3 = v_tiles[jcb].rearrange("p (h d) -> p h d", d=65)
                src = pss[jcb].rearrange("p (h d) -> p h d", d=64)
                nc.vector.tensor_copy(v3[:, c * 8:(c + 1) * 8, 0:64], src)
        # compact K projection (f-major kc tiles [128, JC]);
        # k-half weights batched: one [128, 8*128] load per kfb
        wq4 = t_in["wqkp"].rearrange("(kt p) d -> p kt d", p=128)
        for kfb in range(NT):
            wk = wk2p.tile([128, NT, 128], F32R, tag="wk2", name=f"w_k2{kfb}")
            nc.sync.dma_start(
                out=wk, in_=wq4[:, :, kfb * 256 + 128:kfb * 256 + 256])
            ps = kcpsp.tile([128, N], F32, tag="kcps", name=f"ps_kc{kfb}")
            for kt in range(NT):
                for c2 in range(2):
                    nc.tensor.matmul(
                        ps[:, c2 * 512:c2 * 512 + 320],
                        wk[:, kt, :],
                        xcts[kt][:, c2 * 320:(c2 + 1) * 320],
                        start=(kt == 0), stop=(kt == NT - 1),
                    )
            kc = kcp.tile([128, JC], F32R, tag="kc", name=f"kc{kfb}")
            nc.vector.tensor_copy(kc[:, 0:320], ps[:, 0:320])
            nc.vector.tensor_copy(kc[:, 320:640], ps[:, 512:832])
            kc_tiles.append(kc)
            if kfb == 0 and "dbg_kc" in t_out:
                nc.gpsimd.dma_start(out=t_out["dbg_kc"], in_=kc)
                nc.gpsimd.dma_start(out=t_out["dbg_v"], in_=v_tiles[0])

    def head_loop(ptp, wqp, projp, dpp, Ppp, pep):
        wq4 = t_in["wqkp"].rearrange("(kt p) d -> p kt d", p=128)

        def load_w(fpair, eng):
            parts = []
            for g in range(2):
                w = wqp.tile([128, 4, 256], F32R, tag="wq",
                             name=f"wq{fpair}{g}")
                eng.dma_start(
                    out=w, in_=wq4[:, g * 4:(g + 1) * 4,
                                   fpair * 256:(fpair + 1) * 256])
                parts.append(w)
            return parts

        def proj_ops(fpair, ws):
            """Flat op list: 4 sequential [128,512] chunk accumulations."""
            ops = []
            for half in (0, 1):
                ft = fpair + half * NT
                qk_tiles[ft] = qkp.tile([128, N], F32R, tag="qk",
                                        name=f"qk{ft}")
            for half in (0, 1):
                ft = fpair + half * NT
                for c in (0, 1):
                    ps = projp.tile([128, 512], F32, tag="pj",
                                    name=f"pj{ft}{c}")
                    for kt in range(NT):
                        ops.append(lambda ps=ps, kt=kt, half=half, c=c:
                                   nc.tensor.matmul(
                                       ps[:],
                                       ws[kt // 4][:, kt % 4,
                                                   half * 128:(half + 1) * 128],
                                       xts[kt][:, c * 512:(c + 1) * 512],
                                       start=(kt == 0), stop=(kt == NT - 1)))
                    ops.append(lambda ps=ps, ft=ft, c=c:
                               nc.vector.tensor_copy(
                                   qk_tiles[ft][:, c * 512:(c + 1) * 512], ps))
            return ops

        def pooled_early(it):
            """Matmul ops for pooled attention tile `it` into two proj-pool
            chunks; returns (ops, finish) where finish() emits exp+norm+store.
            """
            chunks = [projp.tile([128, 512], F32, tag="pj",
                                 name=f"pool{it}{c}") for c in range(2)]
            ops = []
            for c in range(2):
                for ft in range(NT):
                    ops.append(lambda c=c, ft=ft: nc.tensor.matmul(
                        chunks[c][:],
                        qk_tiles[ft][:, it * 128:(it + 1) * 128],
                        qk_tiles[NT + ft][:, c * 512:(c + 1) * 512],
                        start=(ft == 0), stop=(ft == NT - 1)))

            def finish():
                pe = pep.tile([128, N], F32, tag="pe", name=f"pe{it}")
                s0 = smallp.tile([128, 2], F32, tag="sm2", bufs=2,
                                 name=f"s2{it}")
                for c in range(2):
                    nc.scalar.activation(
                        pe[:, c * 512:(c + 1) * 512], chunks[c], EXP,
                        scale=SCALE_P, accum_out=s0[:, c:c + 1])
                sums = smallp.tile([128, 1], F32, tag="sm", name=f"sums{it}")
                nc.vector.tensor_add(sums, s0[:, 0:1], s0[:, 1:2])
                rec = smallp.tile([128, 1], F32, tag="rc", name=f"rcp{it}")
                nc.vector.reciprocal(rec, sums)
                nc.vector.tensor_scalar_mul(pe, pe, rec)
                nc.gpsimd.dma_start(
                    out=t_out["attn"][it * 128:(it + 1) * 128, :], in_=pe)
            return ops, finish

        ws = load_w(0, nc.gpsimd)
        pend = proj_ops(0, ws)
        while pend:
            pend.pop(0)()

        finishers = []
        for hp in range(NT):
            finishers = []
            if hp + 1 < NT:
                ws = load_w(hp + 1, nc.sync)
                pend = proj_ops(hp + 1, ws)
            else:
                pend = []
                for it_e in (0, 1):
                    ops, fin = pooled_early(it_e)
                    pend += ops
                    finishers.append(fin)

            # dots + exp stretch, proj(F+1) matmuls woven in
            pts = {}
            for s in range(2 * JT):
                jt, u = divmod(s, 2)
                dp = dpp.tile([128, N], F32, tag="dp", name=f"dp{hp}{u}{jt}")
                off = u * 64
                for c in range(2):
                    nc.tensor.matmul(
                        dp[:, c * 512:(c + 1) * 512],
                        kc_tiles[hp][off:off + 64, jt * 128:(jt + 1) * 128],
                        qk_tiles[hp][off:off + 64, c * 512:(c + 1) * 512],
                        start=True, stop=True,
                    )
                pt = ptp.tile([128, N], BF16, tag="pt", name=f"pt{hp}{u}{jt}")
                nc.scalar.activation(
                    pt, dp, EXP, bias=cmask_t[:, jt:jt + 1], scale=SCALE_H)
                pts[(u, jt)] = pt
                if hp == 0 and s == 0 and "dbg_pt" in t_out:
                    nc.gpsimd.dma_start(out=t_out["dbg_pt"], in_=pt)
                for _ in range(4):
                    if pend:
                        pend.pop(0)()
            while pend:
                pend.pop(0)()
            for fin in finishers:
                fin()

            # attn@V on resident pt tiles + per-partition normalization
            for it in range(NT):
                P = Ppp.tile([128, 512], F32, tag="P", name=f"P{hp}{it}")
                for jt in range(JT):
                    for u in (0, 1):
                        h = 2 * hp + u
                        nc.tensor.matmul(
                            P[:, u * 65:u * 65 + 65],
                            pts[(u, jt)][:, it * 128:(it + 1) * 128],
                            v_tiles[jt][:, h * 65:(h + 1) * 65],
                            start=(jt == 0 and u == 0),
                            stop=(jt == JT - 1 and u == 1),
                        )
                if hp == 0 and it == 0 and "dbg_P" in t_out:
                    dbgp = smallp.tile([128, 512], F32, tag="dbgp", bufs=1,
                                       name="dbgp")
                    nc.vector.tensor_copy(dbgp, P)
                    nc.gpsimd.dma_start(out=t_out["dbg_P"], in_=dbgp)
                rec = smallp.tile([128, 2], F32, tag="rec", bufs=3,
                                  name=f"rec{hp}{it}")
                nc.vector.reciprocal(rec, P[:, 64:130:65])
                rq = smallp.tile([128, 2], F32, tag="rq", bufs=3,
                                 name=f"rq{hp}{it}")
                nc.vector.tensor_scalar_mul(rq, rec, qkT_t[:, it:it + 1])
                aon = aonp.tile([128, 128], BF16, tag="aon",
                                name=f"aon{hp}{it}")
                for u in (0, 1):
                    nc.vector.tensor_scalar_mul(
                        aon[:, u * 64:(u + 1) * 64],
                        P[:, u * 65:u * 65 + 64],
                        rq[:, u:u + 1])
                aon_tiles[hp][it] = aon

    def phase_pooled(pep, pop, trp, aop):
        def emit_tr(hp):
            tr = trp.tile([128, N], BF16, tag="tr", name=f"tr{hp}")
            for k in range(NT):
                nc.tensor.matmul(
                    tr[:, k * 128:(k + 1) * 128],
                    aon_tiles[hp][k][:], ident[:], is_transpose=True)
            ao = aop.tile([128, N], BF16, tag="ao", name=f"ao{hp}")
            nc.vector.tensor_copy(ao, tr)
            ao_tiles[hp] = ao
            if hp == 0 and "dbg_ao" in t_out:
                nc.gpsimd.dma_start(out=t_out["dbg_ao"], in_=ao)

        emit_tr(0)
        for it in range(2, NT):
            ps = pop.tile([128, N], F32, tag="po", name=f"ps_pool{it}")
            for c in range(2):
                for ft in range(NT):
                    nc.tensor.matmul(
                        ps[:, c * 512:(c + 1) * 512],
                        qk_tiles[ft][:, it * 128:(it + 1) * 128],
                        qk_tiles[NT + ft][:, c * 512:(c + 1) * 512],
                        start=(ft == 0), stop=(ft == NT - 1),
                    )
            pe = pep.tile([128, N], F32, tag="pe", name=f"pe{it}")
            sums = smallp.tile([128, 1], F32, tag="sm", name=f"sums{it}")
            nc.scalar.activation(pe, ps, EXP, scale=SCALE_P, accum_out=sums)
            rec = smallp.tile([128, 1], F32, tag="rc", name=f"rcp{it}")
            nc.vector.reciprocal(rec, sums)
            nc.vector.tensor_scalar_mul(pe, pe, rec)
            nc.gpsimd.dma_start(out=t_out["attn"][it * 128:(it + 1) * 128, :],
                                in_=pe)
            emit_tr(it - 1)
        emit_tr(NT - 1)

    def phase_out(wop, otp, opsp):
        for dh in range(2):
            pss = [opsp.tile([128, 512], F32, tag="ops", name=f"ps_o{dh}{i}")
                   for i in range(NT)]
            for ft in range(NT):
                w = wop.tile([128, 512], BF16, tag="wo", name="w_o")
                nc.scalar.dma_start(
                    out=w, in_=t_in["wout"][ft * 128:(ft + 1) * 128,
                                            dh * 512:(dh + 1) * 512])
                for it in range(NT):
                    nc.tensor.matmul(
                        pss[it][:],
                        ao_tiles[ft][:, it * 128:(it + 1) * 128],
                        w[:],
                        start=(ft == 0), stop=False,
                    )
            for it in range(NT):
                nc.tensor.matmul(
                    pss[it][:],
                    fixl[:, it * 128:(it + 1) * 128],
                    fixr[:, dh * 512:(dh + 1) * 512],
                    start=False, stop=True,
                )
                ot = otp.tile([128, 512], F32, tag="ot", bufs=8,
                              name=f"ot{dh}{it}")
                nc.vector.tensor_copy(ot, pss[it])
                eng = nc.sync if it % 2 == 0 else nc.scalar
                eng.dma_start(
                    out=t_out["out"][it * 128:(it + 1) * 128,
                                     dh * 512:(dh + 1) * 512],
                    in_=ot)

    with (
        tc.tile_pool(name="xc", bufs=8) as xcp,
        tc.tile_pool(name="wv", bufs=2) as wvp,
        tc.tile_pool(name="wk2", bufs=2) as wk2p,
        tc.tile_pool(name="vps", bufs=JT, space="PSUM") as vpsp,
        tc.tile_pool(name="kcps", bufs=1, space="PSUM") as kcpsp,
    ):
        phase_vkc(xcp, wvp, wk2p, vpsp, kcpsp)

    with (
        tc.tile_pool(name="pe", bufs=2) as pep,
        tc.tile_pool(name="ao", bufs=8) as aop,
    ):
        with (
            tc.tile_pool(name="pt", bufs=10) as ptp,
            tc.tile_pool(name="wq", bufs=3) as wqp,
            tc.tile_pool(name="pj", bufs=2, space="PSUM") as projp,
            tc.tile_pool(name="dp", bufs=2, space="PSUM") as dpp,
            tc.tile_pool(name="Pp", bufs=2, space="PSUM") as Ppp,
        ):
            head_loop(ptp, wqp, projp, dpp, Ppp, pep)

        with (
            tc.tile_pool(name="po", bufs=2, space="PSUM") as pop,
            tc.tile_pool(name="tr", bufs=2, space="PSUM") as trp,
        ):
            phase_pooled(pep, pop, trp, aop)

        with (
            tc.tile_pool(name="wo", bufs=3) as wop,
            tc.tile_pool(name="ott", bufs=4) as otp,
            tc.tile_pool(name="ops", bufs=8, space="PSUM") as opsp,
        ):
            phase_out(wop, otp, opsp)


def build_nc():
    nc = bacc.Bacc("TRN2", target_bir_lowering=False, debug=False,
                   num_devices=8)
    t_in = {
        "xT": nc.dram_tensor("xT", [D, N], F32R, kind="ExternalInput").ap(),
        "xcT": nc.dram_tensor("xcT", [D, JC], F32R, kind="ExternalInput").ap(),
        "wqkp": nc.dram_tensor("wqkp", [D, 2 * D], F32R,
                               kind="ExternalInput").ap(),
        "wv_in": nc.dram_tensor("wv_in", [D, D], F32R,
                                kind="ExternalInput").ap(),
        "wout": nc.dram_tensor("wout", [D, D], BF16,
                               kind="ExternalInput").ap(),
        "cmaskTc": nc.dram_tensor("cmaskTc", [128, JT], F32,
                                  kind="ExternalInput").ap(),
        "qkeepT": nc.dram_tensor("qkeepT", [128, NT], F32,
                                 kind="ExternalInput").ap(),
        "fixl_in": nc.dram_tensor("fixl_in", [2, N], BF16,
                                  kind="ExternalInput").ap(),
        "fixr_in": nc.dram_tensor("fixr_in", [2, N], BF16,
                                  kind="ExternalInput").ap(),
        "ident_in": nc.dram_tensor("ident_in", [128, 128], BF16,
                                   kind="ExternalInput").ap(),
    }
    t_out = {
        "out": nc.dram_tensor("out", [N, D], F32, kind="ExternalOutput").ap(),
        "attn": nc.dram_tensor("attn", [N, N], F32,
                               kind="ExternalOutput").ap(),
    }
    if os.environ.get("ATTN_DEBUG"):
        t_out["dbg_kc"] = nc.dram_tensor(
            "dbg_kc", [128, JC], F32R, kind="ExternalOutput").ap()
        t_out["dbg_v"] = nc.dram_tensor(
            "dbg_v", [128, 65 * H], BF16, kind="ExternalOutput").ap()
        t_out["dbg_pt"] = nc.dram_tensor(
            "dbg_pt", [128, N], BF16, kind="ExternalOutput").ap()
        t_out["dbg_P"] = nc.dram_tensor(
            "dbg_P", [128, 512], F32, kind="ExternalOutput").ap()
        t_out["dbg_ao"] = nc.dram_tensor(
            "dbg_ao", [128, N], BF16, kind="ExternalOutput").ap()
    with tile.TileContext(
            nc, trace_sim=bool(os.environ.get('ATTN_TRACE_SIM'))) as tc:
        with ExitStack() as es:
            _body(nc, tc, es, t_in, t_out)
    nc.compile()
    return nc


def _bf16(a):
    import ml_dtypes
    return np.asarray(a, np.float32).astype(ml_dtypes.bfloat16)


def _host_prep(x, mask, w_qkv, w_out, b_out):
    wq3 = w_qkv.reshape(D, 3, NT, 128)
    wqkp = np.ascontiguousarray(
        np.stack([wq3[:, 0], wq3[:, 1]], axis=2).reshape(D, 2 * D))
    wv = np.ascontiguousarray(w_qkv[:, 2 * D:])
    wout_b = _bf16(w_out)
    ident = _bf16(np.eye(128, dtype=np.float32))
    in_maps = []
    for b in range(B):
        m = np.concatenate([[True], mask[b]])               # [N]
        keep = np.nonzero(m)[0]
        nk = len(keep)
        assert nk <= JC, f"keep count {nk} exceeds JC={JC}"
        idx = np.concatenate([keep, np.zeros(JC - nk, np.int64)])
        cm_c = np.where(np.arange(JC) < nk, 0.0, NEG).astype(np.float32)
        xTb = np.ascontiguousarray(x[b].T)                  # [D, N]
        xcT = np.ascontiguousarray(xTb[:, idx])             # [D, JC]
        mf = m.astype(np.float32)
        ybar = (x[b].mean(axis=0) @ wv) @ w_out             # [D]
        in_maps.append({
            "xT": xTb,
            "xcT": xcT,
            "wqkp": wqkp,
            "wv_in": wv,
            "wout": wout_b,
            "cmaskTc": np.ascontiguousarray(cm_c.reshape(JT, 128).T),
            "qkeepT": np.ascontiguousarray(mf.reshape(NT, 128).T),
            "fixl_in": _bf16(np.stack([np.ones(N, np.float32), 1.0 - mf])),
            "fixr_in": _bf16(np.stack([b_out, ybar])),
            "ident_in": ident,
        })
    return in_maps


def kernel(x, mask, w_qkv, w_out, b_out, **run_kw):
    if "nc" not in _CACHED:
        _CACHED["nc"] = build_nc()
    nc = _CACHED["nc"]
    in_maps = _host_prep(
        np.asarray(x, np.float32), np.asarray(mask),
        np.asarray(w_qkv, np.float32), np.asarray(w_out, np.float32),
        np.asarray(b_out, np.float32))
    try:
        res = bass_utils.run_bass_kernel_spmd(
            nc, in_maps, core_ids=list(range(B)), **run_kw)
    except Exception:
        # transient NRT device wedge: retry once
        res = bass_utils.run_bass_kernel_spmd(
            nc, in_maps, core_ids=list(range(B)), **run_kw)
    out = np.stack([res.results[b]["out"] for b in range(B)])
    attn_ = np.stack([res.results[b]["attn"] for b in range(B)])
    _CACHED["last_results"] = res
    return out, attn_
